# revision 27
# baseline (speedup 1.0000x reference)
"""DeltaEncoder (delta -> BatchNorm(eval) -> Linear(1,O) -> LIF scan over O) on 8 TRN2 cores.

Strategy (pure data parallel over batch B=32 -> 4 per core):
  * Host folds BN (eval) + Linear(1,O) + the 1/TAU charge factor into per-o
    scalars  A[o] = inv*w[o]/TAU,  C[o] = ((bn_b - mu*inv)*w[o] + b[o])/TAU,
    so the per-step membrane charge is  h = (1-1/TAU)*v + (delta*A[o] + C[o]).
  * Host pre-transposes the per-core input to [(b f), t] so the device sees
    elements as [128 partitions = (b%2, f), free = (b//2, t)] with t contiguous.
  * Device computes delta along t once, then runs the 64-step LIF scan with a
    single fused custom DVE instruction per step:
        h' = (h < 1) * (h * (1-1/TAU)) + (delta * A_o + C_o)
    (state update incl. hard reset + charge, one pass at 1 elem/lane/cycle;
    fp32 tensor_tensor-class ops have no 2x DVE mode, so 64 x ~1.13us of DVE
    streaming is the kernel's floor). Steps of a store group write adjacent
    NFREE-slots of one h-tile in a 3-deep ring.
  * Spikes are extracted per 2-step slice by the scalar engine,
    s = sigmoid(2^100*(h-1)) (saturates to exactly 0.0/1.0), with fp8_e4m3
    output -- 1 byte/spike, 4x less store DMA than f32 -- the final slice on
    the DVE itself (is_ge) to shorten the tail. One contiguous DMA per group
    stores into a blocked DRAM layout [p=(b1,f), o, g, t]; the host permutes
    back to [B, O, F, T] (fp8 -> f32 via byte != 0, exact).
  * Production modules are built WITHOUT TileContext (_build_module_raw):
    semaphores only on true cross-engine edges, none inside the DVE chain --
    Tile's per-op self-waits cost ~104ns/op. Same-engine write->read safety
    comes from stream-length slack (see comments); short producers are
    hoisted before the input-DMA wait. The Tile builder remains for bench
    variants (reps loop, scanonly/noscan bisection).
  * Ramp: the input loads as two g-half DMAs issued from the ACT HWDGE
    queue (its sequencer starts ~0.9us before SP's); each half's delta sub
    and first scan step run as soon as that half lands. Epilogue barrier
    skips the (unused) PE engine. NTFF-measured on HW 2026-08-10: max-core
    88.2us (from 89.2us before these ramp/epilogue changes; DVE scan chain
    is the critical path: 63x1135ns fused LIF steps, zero stalls).
Output: float32 spikes [B, O, F, T].
"""

import numpy as np

# problem shapes (hardcoded per contract)
_B, _T, _F, _O = 32, 512, 64, 64
_NC = 8
_BL = _B // _NC          # 4 batches per core
_G = (_BL * _F) // 128   # 2 free-dim groups of 128 (b,f) rows
_P = 128
_TAU = 2.0
_EPS = 1e-5

_LIF_OP_NAME = "LIF_STEP_ANT_RT"

# which engine extracts spikes: "gpsimd" | "dve" | "act"
SPIKE_ENGINE = "act"
STEPS_PER_DMA = 8
# explicit group-size override (list summing to _O); None -> derived taper
GROUPS_OVERRIDE = None
# number of spike-extract ops per store group (1 = one ACT op per group)
EXTRACT_SPLIT = 4
# use the raw (no TileContext) builder for the production act/fp8 path:
# same dataflow, but no per-op self-semaphores on the DVE chain (engine
# program order already guarantees same-engine RAW; each op's first read
# trails the previous op's first write by a full 1024-cycle stream)
RAW = True
# raw builder: extract the final group's last slice on DVE (is_ge, 2x mode)
# instead of ACT — shortens the kernel tail
RAW_TAIL_DVE = True
# fuse the final LIF step and its spike compare into one custom DVE op
# (the last step's membrane has no consumer, so it needn't materialize).
# Disabled: the runtime walrus pipeline (birverifier..codegen with
# --enable-birsim=true) reproducibly rejects the extra DVE-table row even
# though offline nc.compile() accepts it — likely birsim vs the trailing
# bool (>=) output stage. The select(h>=1, One, Zero) spelling fails the
# runtime pass list too. Worth ~0.6us of kernel tail if the toolchain
# gains support; both spellings verified blocked 2026-08-08.
RAW_TAIL_FUSED = False
# raw builder: emit the epilogue barrier + semaphore clear (needed for safe
# NEFF re-execution; disable only for timing bisection)
RAW_EPILOGUE = True
# engine for the small memsets (d t=0 column, sigmoid bias)
MEMSET_ENGINE = "gpsimd"
# split the input DMA + delta into this many chunks along g (pipeline rampup)
INPUT_SPLIT = 1
HPOOL_BUFS = 3
SPOOL_BUFS = 3
# bake the folded per-o scalars into instruction immediates (saves ~7us/call
# at the cost of a content-keyed NEFF compile on first use)
USE_IMM = True
# raw builder: number of free-dim columns whose LIF chain runs on the GPSIMD
# (Pool) engine instead of the DVE (0 = all on DVE). Measured on HW
# 2026-08-10: CATASTROPHIC — 2.6x slower (Q7 TensorScalarPtr runs ~5 cyc/elem,
# not the model's 1.4) and intermittently WRONG (Pool same-engine W->R
# pipelining race). Keep 0; code path retained for reference only.
GP_COLS = 0
# raw builder: issue the second input-DMA half from the ACT queue so both
# halves transfer in parallel (SP+ACT HWDGEs) instead of serializing on SP
PAR_INPUT_DMA = False
# raw builder: split the input DMA + delta + first scan step by g-half and
# issue the input DMAs from an early-starting queue. True: g0=ACT g1=SP;
# "act2": both ACT; "gp": both GPSIMD (SWDGE ~1us fixed vs HWDGE ~2.1us)
RAMP_GSPLIT = "act2"
# raw builder: issue the final group's store DMA from the GPSIMD SWDGE.
# Measured WORSE (+3us) on HW — SWDGE issue is slower in practice. Keep off.
FINAL_STORE_GP = False
# raw builder: store the final group per extract-slice. Measured WORSE
# (+2.3us) on HW — every extra store DMA costs ~1us of SP/DGE pipeline.
FINAL_STORE_SPLIT = False
# raw builder: barrier only the engines the program uses (no PE): -0.3us
BARRIER_NO_PE = True
# spike store dtype on device: "fp8" (float8_e4m3, 1 byte -- 0.0/1.0 exact,
# host widens via byte != 0) | "bf16" | "f32".
SPIKE_DTYPE = "fp8"

_MODULE_CACHE = {}


def _register_lif_op():
    """Register the fused LIF-step custom DVE op (idempotent)."""
    import concourse.dve_ops as dve_ops
    from concourse.dve_spec import (
        C0, C1, C2, One, Spec, Src0, Src1, _has_src1, lower,
    )
    from concourse.dve_uop import DveOpSpec

    for op in dve_ops.OPS:
        if op.name == _LIF_OP_NAME:
            return op

    def _ref(in0, in1, s0, s1, imm2):
        in0 = np.asarray(in0, np.float32)
        in1 = np.asarray(in1, np.float32)
        keep = (in0 < np.float32(1.0)).astype(np.float32)
        return (
            keep * (in0 * np.float32(imm2))
            + (in1 * np.float32(s0) + np.float32(s1))
        ).astype(np.float32)

    body = (Src0 < One) * (Src0 * C2) + (Src1 * C0 + C1)
    spec = Spec(body=body, reference=_ref)

    row = dve_ops._CUSTOM_DVE_ROW_BASE + len(dve_ops.OPS)
    assert row < 0x20, "no free custom-DVE opcode rows"
    shas = {}
    for ver in ("v3", "v4"):
        uops = lower(spec, ver=ver)
        shas[ver] = DveOpSpec(
            name=_LIF_OP_NAME, opcode=row, uops=uops, rd1_en=_has_src1(spec)
        ).sha(ver)

    op = dve_ops.DveOp(_LIF_OP_NAME, spec, subdim=False, uops_sha=shas)
    dve_ops.OPS.append(op)
    dve_ops._SUB_OPCODE_FOR_NAME[op.name] = row
    dve_ops.CUSTOM_DVE_SPECS[op.name] = spec
    return op


_LIF_SPIKE_OP_NAME = "LIF_STEP_SPIKE_ANT_RT"


def _register_lif_spike_op():
    """Fused final step: spike = (LIF_step(h, d) >= 1), membrane discarded.
    Same inner expression tree as LIF_STEP_ANT_RT (bit-identical h before
    the compare), one extra ALU stage for >= 1."""
    import concourse.dve_ops as dve_ops
    from concourse.dve_spec import (
        C0, C1, C2, One, Spec, Src0, Src1, _has_src1, lower,
    )
    from concourse.dve_uop import DveOpSpec

    for op in dve_ops.OPS:
        if op.name == _LIF_SPIKE_OP_NAME:
            return op

    def _ref(in0, in1, s0, s1, imm2):
        in0 = np.asarray(in0, np.float32)
        in1 = np.asarray(in1, np.float32)
        keep = (in0 < np.float32(1.0)).astype(np.float32)
        h = keep * (in0 * np.float32(imm2)) + (
            in1 * np.float32(s0) + np.float32(s1)
        )
        return (h >= np.float32(1.0)).astype(np.float32)

    body = ((Src0 < One) * (Src0 * C2) + (Src1 * C0 + C1)) >= One
    spec = Spec(body=body, reference=_ref)

    row = dve_ops._CUSTOM_DVE_ROW_BASE + len(dve_ops.OPS)
    assert row < 0x20, "no free custom-DVE opcode rows"
    shas = {}
    for ver in ("v3", "v4"):
        uops = lower(spec, ver=ver)
        shas[ver] = DveOpSpec(
            name=_LIF_SPIKE_OP_NAME, opcode=row, uops=uops,
            rd1_en=_has_src1(spec)
        ).sha(ver)

    op = dve_ops.DveOp(_LIF_SPIKE_OP_NAME, spec, subdim=False, uops_sha=shas)
    dve_ops.OPS.append(op)
    dve_ops._SUB_OPCODE_FOR_NAME[op.name] = row
    dve_ops.CUSTOM_DVE_SPECS[op.name] = spec
    return op


def _spike_groups():
    """Store/extract group sizes along o: SPD-step groups, tapered at the
    end to shrink the kernel tail (small final ACT + DMA)."""
    if GROUPS_OVERRIDE is not None:
        assert sum(GROUPS_OVERRIDE) == _O
        return list(GROUPS_OVERRIDE)
    groups = []
    rem = _O
    while rem > 2 * STEPS_PER_DMA:
        groups.append(STEPS_PER_DMA)
        rem -= STEPS_PER_DMA
    while rem > 0:
        g_sz = rem if rem <= 2 else max(2, rem // 2)
        groups.append(g_sz)
        rem -= g_sz
    return groups


def _build_module(
    spike_engine: str, reps: int = 1, variant: str = "full", imm_coefs=None
):
    """Build the Bass/Tile module (one NeuronCore program, SPMD across 8).

    reps > 1 wraps the whole computation in a device-side loop -- used only
    for differential wall-clock timing (output is rewritten identically).
    variant: "full" | "scanonly" (no spikes/stores) | "nostore" (no DMA out)
             | "noscan" (one memset h + spikes/stores only) -- bench-only.
    """
    import concourse.bacc as bacc
    import concourse.mybir as mybir
    from concourse.tile import TileContext

    lif_op = _register_lif_op()

    nc = bacc.Bacc(
        "TRN2",
        target_bir_lowering=False,
        debug=False,
        enable_asserts=False,
        num_devices=_NC,
    )
    f32 = mybir.dt.float32

    NFREE = _G * _T          # 1024
    groups = _spike_groups()

    x_in = nc.dram_tensor("x_bft", [_BL * _F, _T], f32, kind="ExternalInput").ap()
    a_in = nc.dram_tensor("a_coef", [_P, _O], f32, kind="ExternalInput").ap()
    c_in = nc.dram_tensor("c_coef", [_P, _O], f32, kind="ExternalInput").ap()
    if SPIKE_DTYPE == "fp8":
        sdt = mybir.dt.float8e4
    elif SPIKE_DTYPE == "bf16":
        sdt = mybir.dt.bfloat16
    else:
        sdt = f32
    # blocked layout: [p=(b1,f), o, g, t]; host permutes to [b=2g+b1, o, f, t]
    out = nc.dram_tensor(
        "spikes", [_P, _O, _G, _T], sdt, kind="ExternalOutput"
    ).ap()
    out2 = out.rearrange("p o g t -> p (o g t)")  # [128, O*NFREE], contiguous rows

    with TileContext(nc) as tc:
        with (
            tc.tile_pool(name="const", bufs=1) as cpool,
            tc.tile_pool(name="xd", bufs=1) as xpool,
            tc.tile_pool(name="state", bufs=HPOOL_BUFS) as hpool,
            tc.tile_pool(name="spk", bufs=SPOOL_BUFS) as spool,
        ):

            def body():
                if variant == "empty":
                    z_t = cpool.tile([_P, 1], f32, tag="z")
                    nc.vector.memset(z_t[:], 0.0)
                    return
                a_t = c_t = None
                if imm_coefs is None:
                    a_t = cpool.tile([_P, _O], f32, tag="a")
                    c_t = cpool.tile([_P, _O], f32, tag="c")
                    nc.sync.dma_start(out=a_t[:], in_=a_in[:])
                    nc.sync.dma_start(out=c_t[:], in_=c_in[:])

                x_t = xpool.tile([_P, NFREE], f32, tag="x")
                d_t = xpool.tile([_P, NFREE], f32, tag="d")
                x3 = x_t[:].rearrange("p (g t) -> p g t", g=_G)
                d3 = d_t[:].rearrange("p (g t) -> p g t", g=_G)
                x_src = x_in.rearrange("(g p) t -> p g t", p=_P)
                mset = nc.gpsimd if MEMSET_ENGINE == "gpsimd" else nc.vector
                # delta along t: d[...,0] = 0 ; d[...,1:] = x[...,1:] - x[...,:-1]
                # (input DMA + sub optionally chunked along g for faster rampup)
                if INPUT_SPLIT <= 1:
                    nc.sync.dma_start(out=x3, in_=x_src)
                    mset.memset(d3[:, :, 0:1], 0.0)
                    nc.vector.tensor_sub(
                        out=d3[:, :, 1:_T], in0=x3[:, :, 1:_T],
                        in1=x3[:, :, 0 : _T - 1],
                    )
                else:
                    for gi in range(_G):
                        nc.sync.dma_start(
                            out=x3[:, gi : gi + 1], in_=x_src[:, gi : gi + 1]
                        )
                    mset.memset(d3[:, :, 0:1], 0.0)
                    for gi in range(_G):
                        nc.vector.tensor_sub(
                            out=d3[:, gi : gi + 1, 1:_T],
                            in0=x3[:, gi : gi + 1, 1:_T],
                            in1=x3[:, gi : gi + 1, 0 : _T - 1],
                        )

                sigb = None
                if spike_engine == "act" and variant != "scanonly":
                    sigb = cpool.tile([_P, 1], f32, tag="sigb")
                    mset.memset(sigb[:], -(2.0**100))
                h_zero = None
                if variant == "noscan":
                    # one static h tile reused by every group's extraction
                    h_zero = cpool.tile([_P, STEPS_PER_DMA * NFREE], f32, tag="hz")
                    nc.vector.memset(h_zero[:], 0.0)

                decay = 1.0 - 1.0 / _TAU
                o_base = 0
                h_half = None   # AP of the previous step's h slot
                for g_sz in groups:
                    if variant != "noscan":
                        h_g = hpool.tile([_P, g_sz * NFREE], f32, tag="h")
                        for oi in range(g_sz):
                            o = o_base + oi
                            out_ap = h_g[:, oi * NFREE : (oi + 1) * NFREE]
                            if imm_coefs is not None:
                                s0o, s1o = float(imm_coefs[0][o]), float(imm_coefs[1][o])
                            else:
                                s0o, s1o = a_t[:, o : o + 1], c_t[:, o : o + 1]
                            if o == 0:
                                # v=0: h_0 = d*A_0 + C_0 (2x-mode tensor_scalar,
                                # replaces state memset + first custom op)
                                nc.vector.tensor_scalar(
                                    out_ap,
                                    d_t[:],
                                    s0o,
                                    s1o,
                                    mybir.AluOpType.mult,
                                    mybir.AluOpType.add,
                                )
                            else:
                                nc.vector._custom_dve(
                                    lif_op,
                                    out=out_ap,
                                    in0=h_half,
                                    in1=d_t[:],
                                    s0=s0o,
                                    s1=s1o,
                                    imm2=decay,
                                )
                            h_half = out_ap
                    else:
                        h_g = h_zero
                    if variant == "scanonly":
                        o_base += g_sz
                        continue
                    s_mega = spool.tile([_P, g_sz * NFREE], sdt, tag="s")
                    nsp = min(EXTRACT_SPLIT, g_sz)
                    bounds = [g_sz * i // nsp for i in range(nsp + 1)]
                    for b0, b1 in zip(bounds[:-1], bounds[1:]):
                        s_ap = s_mega[:, b0 * NFREE : b1 * NFREE]
                        h_ap = h_g[:, b0 * NFREE : b1 * NFREE]
                        if spike_engine == "gpsimd":
                            nc.gpsimd.tensor_single_scalar(
                                s_ap, h_ap, 1.0, mybir.AluOpType.is_ge
                            )
                        elif spike_engine == "act":
                            # sigmoid(2^100*(h-1)) saturates to exactly 0/1
                            nc.scalar.activation(
                                s_ap,
                                h_ap,
                                mybir.ActivationFunctionType.Sigmoid,
                                bias=sigb[:],
                                scale=2.0**100,
                            )
                        else:
                            nc.vector.tensor_single_scalar(
                                s_ap, h_ap, 1.0, mybir.AluOpType.is_ge
                            )
                    if variant != "nostore":
                        lo = o_base * NFREE
                        hi = (o_base + g_sz) * NFREE
                        nc.sync.dma_start(out=out2[:, lo:hi], in_=s_mega[:])
                    o_base += g_sz

            if reps == 1:
                body()
            else:
                with tc.For_i(0, reps, 1):
                    body()

    nc.finalize()
    return nc


def _build_module_raw(imm_coefs):
    """No-Tile production builder (act engine, fp8 spikes, imm coefs).

    Identical dataflow to _build_module(variant="full"), but semaphores only
    on true cross-engine edges:
      s_x    input DMA done + delta    -> DVE delta / GP scan start
      s_dve  DVE scan progress         -> ACT extract slices
      s_gp   GP scan progress (planes) -> ACT extract slices
      s_act  extracts done             -> SP store DMA + h-ring reuse
      s_dma[j] store DMA (s-slot j)    -> s-ring slot reuse
    The DVE scan chain itself carries no waits: the engine is in-order and
    each op's first read trails the previous op's first write by a full
    stream (>> SBUF write latency).

    GP_COLS > 0 splits each h plane's free dim: cols [0, D) scan on the
    DVE (custom fused op), cols [D, NFREE) scan on the GPSIMD (Pool)
    engine via 4 plain ops/step. Both write disjoint ranges of the same
    h tile, so ACT extraction and the store DMA see whole planes.
    """
    import contextlib

    import concourse.bacc as bacc
    import concourse.mybir as mybir

    assert imm_coefs is not None
    lif_op = _register_lif_op()
    # only register the fused op when used — keeps the DVE table identical
    # to the hardware-validated build otherwise
    lif_spike_op = _register_lif_spike_op() if RAW_TAIL_FUSED else None

    nc = bacc.Bacc(
        "TRN2",
        target_bir_lowering=False,
        debug=False,
        enable_asserts=False,
        num_devices=_NC,
    )
    f32 = mybir.dt.float32
    sdt = {"fp8": mybir.dt.float8e4, "bf16": mybir.dt.bfloat16,
           "f32": f32}[SPIKE_DTYPE]

    NFREE = _G * _T          # 1024
    GP = GP_COLS
    D = NFREE - GP           # DVE-owned columns per plane
    assert not (RAMP_GSPLIT and GP), "RAMP_GSPLIT writes full-width plane 0"
    groups = _spike_groups()
    if RAMP_GSPLIT:
        # plane 0 is emitted from the prologue without a slice-boundary
        # s_dve inc; the first extract slice must span >= 2 planes
        nsp0 = min(EXTRACT_SPLIT, groups[0])
        assert groups[0] // nsp0 >= 2, "RAMP_GSPLIT needs first slice >= 2"
    ngrp = len(groups)
    NRING = 3                # h / s ring depth
    HMAX = max(groups)

    x_in = nc.dram_tensor("x_bft", [_BL * _F, _T], f32, kind="ExternalInput").ap()
    out = nc.dram_tensor(
        "spikes", [_P, _O, _G, _T], sdt, kind="ExternalOutput"
    ).ap()
    out2 = out.rearrange("p o g t -> p (o g t)")

    s_x = nc.alloc_semaphore("s_x")
    s_x2 = nc.alloc_semaphore("s_x2")
    s_d = nc.alloc_semaphore("s_d") if GP else None
    s_dve = nc.alloc_semaphore("s_dve")
    s_gp = nc.alloc_semaphore("s_gp") if GP else None
    s_act = nc.alloc_semaphore("s_act")
    s_dma = [nc.alloc_semaphore(f"s_dma{j}") for j in range(NRING)]

    with contextlib.ExitStack() as stack:
        xt = stack.enter_context(nc.sbuf_tensor("x", [_P, NFREE], f32))
        dt_ = stack.enter_context(nc.sbuf_tensor("d", [_P, NFREE], f32))
        sgt = stack.enter_context(nc.sbuf_tensor("sigb", [_P, 1], f32))
        hts = [stack.enter_context(
                   nc.sbuf_tensor(f"h{j}", [_P, HMAX * NFREE], f32))
               for j in range(NRING)]
        sts = [stack.enter_context(
                   nc.sbuf_tensor(f"s{j}", [_P, HMAX * NFREE], sdt))
               for j in range(NRING)]
        gp_scr = []
        if GP:
            gp_scr = [stack.enter_context(
                          nc.sbuf_tensor(f"gps{j}", [_P, GP], f32))
                      for j in range(3)]
        x_t, d_t, sigb = xt.ap(), dt_.ap(), sgt.ap()
        h_bufs = [t.ap() for t in hts]
        s_bufs = [t.ap() for t in sts]
        k_t, r_t, e_t = (t.ap() for t in gp_scr) if GP else (None, None, None)

        x3 = x_t.rearrange("p (g t) -> p g t", g=_G)
        d3 = d_t.rearrange("p (g t) -> p g t", g=_G)
        d_dve = d_t[:, 0:D]
        d_gp = d_t[:, D:NFREE] if GP else None

        x_src = x_in.rearrange("(g p) t -> p g t", p=_P)
        TH = _T // 2
        h0_full = None
        if RAMP_GSPLIT:
            # input split by g-half; g0 issues from the ACT HWDGE queue
            # (its sequencer starts ~0.9us before SP's). Delta is per-g
            # along t, so each half's sub + first scan step run as soon as
            # that half lands; the g1 transfer overlaps g0's sub + o=0a.
            eng1 = nc.gpsimd if RAMP_GSPLIT == "gp" else nc.scalar
            eng2 = {"act2": nc.scalar, "gp": nc.gpsimd}.get(
                RAMP_GSPLIT, nc.sync)
            eng1.dma_start(
                out=x3[:, 0:1], in_=x_src[:, 0:1]
            ).then_inc(s_x, 16)
            eng2.dma_start(
                out=x3[:, 1:2], in_=x_src[:, 1:2]
            ).then_inc(s_x2, 16)
            # memsets before the DMA wait: the wait gives their SBUF writes
            # commit slack (same-engine W->R has no interlock)
            nc.vector.memset(d3[:, :, 0:1], 0.0)
            nc.vector.memset(sigb, -(2.0**100))
            s00 = float(imm_coefs[0][0])
            s10 = float(imm_coefs[1][0])
            h_g0 = h_bufs[0]
            nc.vector.wait_ge(s_x, 16)
            nc.vector.tensor_sub(
                out=d3[:, 0:1, 1:_T], in0=x3[:, 0:1, 1:_T],
                in1=x3[:, 0:1, 0 : _T - 1],
            )
            # o=0 on the g0 half (tensor_scalar, 2x_2p) while g1 transfers
            nc.vector.tensor_scalar(
                h_g0[:, 0:_T], d_t[:, 0:_T], s00, s10,
                mybir.AluOpType.mult, mybir.AluOpType.add,
            )
            nc.vector.wait_ge(s_x2, 16)
            nc.vector.tensor_sub(
                out=d3[:, 1:2, 1:_T], in0=x3[:, 1:2, 1:_T],
                in1=x3[:, 1:2, 0 : _T - 1],
            )
            ins = nc.vector.tensor_scalar(
                h_g0[:, _T:NFREE], d_t[:, _T:NFREE], s00, s10,
                mybir.AluOpType.mult, mybir.AluOpType.add,
            )
            h0_full = h_g0[:, 0:NFREE]
        else:
            # input DMA split along t; with PAR_INPUT_DMA the second half
            # issues from the ACT HWDGE so both halves transfer concurrently
            nc.sync.dma_start(
                out=x3[:, :, 0:TH], in_=x_src[:, :, 0:TH]
            ).then_inc(s_x, 16)
            eng2 = nc.scalar if PAR_INPUT_DMA else nc.sync
            eng2.dma_start(
                out=x3[:, :, TH:_T], in_=x_src[:, :, TH:_T]
            ).then_inc(s_x2, 16)

            # DVE prologue. Both memsets go FIRST, before the input-DMA
            # wait: the engine has no same-engine write->read interlock, so
            # a short producer's SBUF write (~58-116 cycles to commit) must
            # be given slack before its consumer's first read. The multi-us
            # DMA wait provides it. (The LIF chain needs no such care: op
            # k+1's read of element i trails op k's write by a full stream.)
            nc.vector.memset(d3[:, :, 0:1], 0.0)
            nc.vector.memset(sigb, -(2.0**100))
            nc.vector.wait_ge(s_x, 16)
            nc.vector.tensor_sub(
                out=d3[:, :, 1:TH], in0=x3[:, :, 1:TH],
                in1=x3[:, :, 0 : TH - 1],
            )
            nc.vector.wait_ge(s_x2, 16)
            # second half includes the boundary column t=TH (reads x[TH-1]
            # from the first half's region — cross-DMA, both sem-gated)
            ins = nc.vector.tensor_sub(
                out=d3[:, :, TH:_T], in0=x3[:, :, TH:_T],
                in1=x3[:, :, TH - 1 : _T - 1],
            )
        if GP:
            # d fully materialized -> GP scan may start (dedicated sem: the
            # input-DMA incs on s_x would satisfy a combined threshold early)
            ins.then_inc(s_d, 1)
            nc.gpsimd.wait_ge(s_d, 1)

        # static schedule bookkeeping
        decay = 1.0 - 1.0 / _TAU
        dve_incs = 0          # s_dve increments issued so far
        act_incs = 0          # s_act increments issued so far
        dma_cnt = [0] * NRING  # store DMAs issued per s-ring slot
        # per-group precomputed: extract slice bounds
        o_base = 0
        h_half = None         # DVE carry AP (cols [0:D) of prev plane)
        gp_half = None        # GP carry AP (cols [D:NFREE) of prev plane)
        acts_after_group = []
        alu = mybir.AluOpType
        for gi, g_sz in enumerate(groups):
            h_g = h_bufs[gi % NRING]
            s_g = s_bufs[gi % NRING]
            nsp = min(EXTRACT_SPLIT, g_sz)
            bounds = [g_sz * i // nsp for i in range(nsp + 1)]
            if gi >= NRING:
                # h-ring reuse: ACT must have finished reading group gi-NRING
                nc.vector.wait_ge(s_act, acts_after_group[gi - NRING])
                if GP:
                    nc.gpsimd.wait_ge(s_act, acts_after_group[gi - NRING])
            for oi in range(g_sz):
                o = o_base + oi
                s0o = float(imm_coefs[0][o])
                s1o = float(imm_coefs[1][o])
                out_ap = h_g[:, oi * NFREE : oi * NFREE + D]
                if o == 0 and RAMP_GSPLIT:
                    # plane 0 was produced by the prologue's two g-half
                    # tensor_scalar ops (into this same ring slot)
                    h_half = h0_full
                    continue
                if o == 0:
                    ins = nc.vector.tensor_scalar(
                        out_ap, d_dve, s0o, s1o, alu.mult, alu.add,
                    )
                else:
                    ins = nc.vector._custom_dve(
                        lif_op, out=out_ap, in0=h_half, in1=d_dve,
                        s0=s0o, s1=s1o, imm2=decay,
                    )
                h_half = out_ap
                if oi + 1 in bounds:
                    ins.then_inc(s_dve, 1)
                    dve_incs += 1
                if GP:
                    gp_out = h_g[:, oi * NFREE + D : (oi + 1) * NFREE]
                    if o == 0:
                        gins = nc.gpsimd.tensor_scalar(
                            gp_out, d_gp, s0o, s1o, alu.mult, alu.add,
                        )
                    else:
                        # k = 0.5*[h<1]; r = k*h; e = d*A+C; h' = r + e
                        nc.gpsimd.tensor_scalar(
                            k_t, gp_half, 1.0, decay, alu.is_lt, alu.mult,
                        )
                        nc.gpsimd.tensor_tensor(
                            out=r_t, in0=k_t, in1=gp_half, op=alu.mult,
                        )
                        nc.gpsimd.tensor_scalar(
                            e_t, d_gp, s0o, s1o, alu.mult, alu.add,
                        )
                        gins = nc.gpsimd.tensor_tensor(
                            out=gp_out, in0=r_t, in1=e_t, op=alu.add,
                        )
                    gp_half = gp_out
                    gins.then_inc(s_gp, 1)  # planes completed (global o+1)
            # ACT extraction for this group, slice by slice
            for si, (b0, b1) in enumerate(zip(bounds[:-1], bounds[1:])):
                on_dve = (
                    RAW_TAIL_DVE and gi == ngrp - 1 and si == nsp - 1
                )
                if on_dve:
                    # final slice: extract on DVE (and GP for its columns)
                    # right after the last LIF step (program order, no sem
                    # hop; fp8 out). WAR guard: this slot's previous store
                    # DMA must be done (satisfied long before; free).
                    nc.vector.wait_ge(
                        s_dma[gi % NRING], 16 * dma_cnt[gi % NRING]
                    )
                    n_pl = b1 - b0
                    if GP:
                        sl_s = s_g[:, b0 * NFREE : b1 * NFREE].rearrange(
                            "p (n f) -> p n f", n=n_pl)
                        sl_h = h_g[:, b0 * NFREE : b1 * NFREE].rearrange(
                            "p (n f) -> p n f", n=n_pl)
                        nc.vector.tensor_single_scalar(
                            sl_s[:, :, 0:D], sl_h[:, :, 0:D],
                            1.0, alu.is_ge,
                        ).then_inc(s_act, 1)
                        act_incs += 1
                        nc.gpsimd.wait_ge(
                            s_dma[gi % NRING], 16 * dma_cnt[gi % NRING]
                        )
                        nc.gpsimd.tensor_single_scalar(
                            sl_s[:, :, D:NFREE], sl_h[:, :, D:NFREE],
                            1.0, alu.is_ge,
                        ).then_inc(s_act, 1)
                        act_incs += 1
                    else:
                        nc.vector.tensor_single_scalar(
                            s_g[:, b0 * NFREE : b1 * NFREE],
                            h_g[:, b0 * NFREE : b1 * NFREE],
                            1.0, alu.is_ge,
                        ).then_inc(s_act, 1)
                        act_incs += 1
                    continue
                # progress target: all slices up to b1 of this group done
                done_slices = dve_incs - (nsp - 1 - si)
                nc.scalar.wait_ge(s_dve, done_slices)
                if GP:
                    nc.scalar.wait_ge(s_gp, o_base + b1)
                if si == 0 and gi >= NRING:
                    # s-ring reuse: slot's previous store DMA must be done
                    nc.scalar.wait_ge(
                        s_dma[gi % NRING], 16 * dma_cnt[gi % NRING]
                    )
                nc.scalar.activation(
                    s_g[:, b0 * NFREE : b1 * NFREE],
                    h_g[:, b0 * NFREE : b1 * NFREE],
                    mybir.ActivationFunctionType.Sigmoid,
                    bias=sigb,
                    scale=2.0**100,
                ).then_inc(s_act, 1)
                act_incs += 1
            acts_after_group.append(act_incs)
            # store DMA for the whole group (final group optionally via the
            # GPSIMD SWDGE — lower issue latency, shorter tail)
            st_eng = (nc.gpsimd if (FINAL_STORE_GP and gi == ngrp - 1)
                      else nc.sync)
            if FINAL_STORE_SPLIT and gi == ngrp - 1 and nsp > 1:
                # per-slice stores: earlier slices' transfers overlap the
                # final slice's extraction; the last transfer shrinks
                for si, (b0, b1) in enumerate(zip(bounds[:-1], bounds[1:])):
                    st_eng.wait_ge(
                        s_act, act_incs - (nsp - 1 - si) - (1 if GP else 0))
                    st_eng.dma_start(
                        out=out2[:, (o_base + b0) * NFREE
                                 : (o_base + b1) * NFREE],
                        in_=s_g[:, b0 * NFREE : b1 * NFREE],
                    ).then_inc(s_dma[gi % NRING], 16)
                    dma_cnt[gi % NRING] += 1
                o_base += g_sz
                continue
            st_eng.wait_ge(s_act, act_incs)
            lo = o_base * NFREE
            hi = (o_base + g_sz) * NFREE
            st_eng.dma_start(
                out=out2[:, lo:hi], in_=s_g[:, : g_sz * NFREE]
            ).then_inc(s_dma[gi % NRING], 16)
            dma_cnt[gi % NRING] += 1
            o_base += g_sz

        # end of program: every store DMA landed, then reset semaphores so a
        # re-execution of the same NEFF starts from zero. One barrier before
        # the clear (every engine is provably past its final sem wait — a
        # clear racing a polling waiter would deadlock); no second barrier:
        # re-execution is gated on full queue drain, which includes the
        # gpsimd clear itself.
        for j in range(NRING):
            nc.sync.wait_ge(s_dma[j], 16 * dma_cnt[j])
        nc.sync.drain()
        if RAW_EPILOGUE:
            if BARRIER_NO_PE:
                import concourse.mybir as _mb
                nc.multi_engine_barrier(
                    [_mb.EngineType.SP, _mb.EngineType.DVE,
                     _mb.EngineType.Activation, _mb.EngineType.Pool])
            else:
                nc.all_engine_barrier()
            sems = [s_x, s_x2, s_dve, s_act, *s_dma]
            if s_gp is not None:
                sems.extend([s_gp, s_d])
            nc.clear_and_free_semaphores(sems)

    nc.finalize()
    return nc


def _get_module(spike_engine: str, imm_coefs=None):
    raw = RAW and spike_engine == "act" and imm_coefs is not None
    if imm_coefs is not None:
        key = (spike_engine, SPIKE_DTYPE, raw, GP_COLS, PAR_INPUT_DMA,
               str(RAMP_GSPLIT), FINAL_STORE_GP, FINAL_STORE_SPLIT,
               BARRIER_NO_PE,
               imm_coefs[0].tobytes(), imm_coefs[1].tobytes())
    else:
        key = (spike_engine, SPIKE_DTYPE, raw)
    if key not in _MODULE_CACHE:
        if raw:
            _MODULE_CACHE[key] = _build_module_raw(imm_coefs)
        else:
            _MODULE_CACHE[key] = _build_module(spike_engine, imm_coefs=imm_coefs)
    return _MODULE_CACHE[key]


def _prepare_inputs(inputs, enc_w, enc_b, bn_w, bn_b, bn_mean, bn_var):
    """Host-side marshalling: scalar folding + per-core shard/transpose."""
    x = np.ascontiguousarray(np.asarray(inputs, np.float32))
    w = np.asarray(enc_w, np.float32).reshape(_O)
    b = np.asarray(enc_b, np.float32).reshape(_O)
    bw = np.float64(np.asarray(bn_w).reshape(())[()])
    bb = np.float64(np.asarray(bn_b).reshape(())[()])
    bm = np.float64(np.asarray(bn_mean).reshape(())[()])
    bv = np.float64(np.asarray(bn_var).reshape(())[()])

    inv = bw / np.sqrt(bv + _EPS)
    beta = bb - bm * inv
    A = (inv * w.astype(np.float64) / _TAU).astype(np.float32)
    C = (((beta * w.astype(np.float64)) + b.astype(np.float64)) / _TAU).astype(
        np.float32
    )
    a_b = np.ascontiguousarray(np.broadcast_to(A, (_P, _O)))
    c_b = np.ascontiguousarray(np.broadcast_to(C, (_P, _O)))

    in_maps = []
    for core in range(_NC):
        xc = x[core * _BL : (core + 1) * _BL]          # [4, T, F]
        xt = np.ascontiguousarray(xc.transpose(0, 2, 1)).reshape(_BL * _F, _T)
        in_maps.append({"x_bft": xt, "a_coef": a_b, "c_coef": c_b})
    return in_maps


def _to_f32_spikes(v: np.ndarray) -> np.ndarray:
    """Device spike array -> f32 0.0/1.0 (exact: spike encodings are
    0x00 vs nonzero in every supported dtype; sigmoid emits +0.0 only)."""
    v = np.asarray(v)
    if v.dtype == np.float32:
        return v
    if v.itemsize == 1:
        return (v.view(np.uint8) != 0).astype(np.float32)
    if v.itemsize == 2:
        return (v.view(np.uint16) != 0).astype(np.float32)
    raise ValueError(f"unexpected spike dtype {v.dtype}")


def _unpack_core(spk_blocked: np.ndarray) -> np.ndarray:
    """[p=(b1,f), o, g, t] -> [b=2g+b1, o, f, t] (widened to f32)."""
    v = _to_f32_spikes(spk_blocked)
    v = v.reshape(2, _F, _O, _G, _T)                     # [b1, f, o, g, t]
    v = v.transpose(3, 0, 2, 1, 4)                       # [g, b1, o, f, t]
    return np.ascontiguousarray(v.reshape(_BL, _O, _F, _T))


def _run(in_maps, spike_engine=None, **spmd_kwargs):
    from concourse.bass_utils import run_bass_kernel_spmd

    eng = spike_engine or SPIKE_ENGINE
    imm_coefs = None
    if USE_IMM:
        imm_coefs = (in_maps[0]["a_coef"][0], in_maps[0]["c_coef"][0])
    nc = _get_module(eng, imm_coefs)
    return run_bass_kernel_spmd(nc, in_maps, core_ids=list(range(_NC)), **spmd_kwargs)


def kernel(inputs, enc_w, enc_b, bn_w, bn_b, bn_mean, bn_var):
    in_maps = _prepare_inputs(inputs, enc_w, enc_b, bn_w, bn_b, bn_mean, bn_var)
    res = _run(in_maps)
    out = np.concatenate([_unpack_core(r["spikes"]) for r in res.results], axis=0)
    return np.ascontiguousarray(out.astype(np.float32, copy=False))



# revision 31
# speedup vs baseline: 1.0014x; 1.0014x over previous
"""DeltaEncoder (delta -> BatchNorm(eval) -> Linear(1,O) -> LIF scan over O) on 8 TRN2 cores.

Strategy (pure data parallel over batch B=32 -> 4 per core):
  * Host folds BN (eval) + Linear(1,O) + the 1/TAU charge factor into per-o
    scalars  A[o] = inv*w[o]/TAU,  C[o] = ((bn_b - mu*inv)*w[o] + b[o])/TAU,
    so the per-step membrane charge is  h = (1-1/TAU)*v + (delta*A[o] + C[o]).
  * Host pre-transposes the per-core input to [(b f), t] so the device sees
    elements as [128 partitions = (b%2, f), free = (b//2, t)] with t contiguous.
  * Device computes delta along t once, then runs the 64-step LIF scan with a
    single fused custom DVE instruction per step:
        h' = (h < 1) * (h * (1-1/TAU)) + (delta * A_o + C_o)
    (state update incl. hard reset + charge, one pass at 1 elem/lane/cycle;
    fp32 tensor_tensor-class ops have no 2x DVE mode, so 64 x ~1.13us of DVE
    streaming is the kernel's floor). Steps of a store group write adjacent
    NFREE-slots of one h-tile in a 3-deep ring.
  * Spikes are extracted per 2-step slice by the scalar engine,
    s = sigmoid(2^100*(h-1)) (saturates to exactly 0.0/1.0), with fp8_e4m3
    output -- 1 byte/spike, 4x less store DMA than f32 -- the final slice on
    the DVE itself (is_ge) to shorten the tail. One contiguous DMA per group
    stores into a blocked DRAM layout [p=(b1,f), o, g, t]; the host permutes
    back to [B, O, F, T] (fp8 -> f32 via byte != 0, exact).
  * Production modules are built WITHOUT TileContext (_build_module_raw):
    semaphores only on true cross-engine edges, none inside the DVE chain --
    Tile's per-op self-waits cost ~104ns/op. Same-engine write->read safety
    comes from stream-length slack (see comments); short producers are
    hoisted before the input-DMA wait. The Tile builder remains for bench
    variants (reps loop, scanonly/noscan bisection).
  * Ramp: the input loads as two g-half DMAs issued from the ACT HWDGE
    queue (its sequencer starts ~0.9us before SP's); each half's delta sub
    and first scan step run as soon as that half lands. Epilogue barrier
    skips the (unused) PE engine. NTFF-measured on HW 2026-08-10: max-core
    88.2us (from 89.2us before these ramp/epilogue changes; DVE scan chain
    is the critical path: 63x1135ns fused LIF steps, zero stalls).
Output: float32 spikes [B, O, F, T].
"""

import numpy as np

# problem shapes (hardcoded per contract)
_B, _T, _F, _O = 32, 512, 64, 64
_NC = 8
_BL = _B // _NC          # 4 batches per core
_G = (_BL * _F) // 128   # 2 free-dim groups of 128 (b,f) rows
_P = 128
_TAU = 2.0
_EPS = 1e-5

_LIF_OP_NAME = "LIF_STEP_ANT_RT"

# which engine extracts spikes: "gpsimd" | "dve" | "act"
SPIKE_ENGINE = "act"
STEPS_PER_DMA = 8
# explicit group-size override (list summing to _O); None -> derived taper
GROUPS_OVERRIDE = None
# number of spike-extract ops per store group (1 = one ACT op per group)
EXTRACT_SPLIT = 4
# use the raw (no TileContext) builder for the production act/fp8 path:
# same dataflow, but no per-op self-semaphores on the DVE chain (engine
# program order already guarantees same-engine RAW; each op's first read
# trails the previous op's first write by a full 1024-cycle stream)
RAW = True
# raw builder: extract the final group's last slice on DVE (is_ge, 2x mode)
# instead of ACT — shortens the kernel tail
RAW_TAIL_DVE = True
# fuse the final LIF step and its spike compare into one custom DVE op
# (the last step's membrane has no consumer, so it needn't materialize).
# Disabled: the runtime walrus pipeline (birverifier..codegen with
# --enable-birsim=true) reproducibly rejects the extra DVE-table row even
# though offline nc.compile() accepts it — likely birsim vs the trailing
# bool (>=) output stage. The select(h>=1, One, Zero) spelling fails the
# runtime pass list too. Worth ~0.6us of kernel tail if the toolchain
# gains support; both spellings verified blocked 2026-08-08.
RAW_TAIL_FUSED = False
# raw builder: emit the epilogue barrier + semaphore clear (needed for safe
# NEFF re-execution; disable only for timing bisection)
RAW_EPILOGUE = True
# engine for the small memsets (d t=0 column, sigmoid bias)
MEMSET_ENGINE = "gpsimd"
# split the input DMA + delta into this many chunks along g (pipeline rampup)
INPUT_SPLIT = 1
HPOOL_BUFS = 3
SPOOL_BUFS = 3
# bake the folded per-o scalars into instruction immediates (saves ~7us/call
# at the cost of a content-keyed NEFF compile on first use)
USE_IMM = True
# raw builder: number of free-dim columns whose LIF chain runs on the GPSIMD
# (Pool) engine instead of the DVE (0 = all on DVE). Measured on HW
# 2026-08-10: CATASTROPHIC — 2.6x slower (Q7 TensorScalarPtr runs ~5 cyc/elem,
# not the model's 1.4) and intermittently WRONG (Pool same-engine W->R
# pipelining race). Keep 0; code path retained for reference only.
GP_COLS = 0
# raw builder: issue the second input-DMA half from the ACT queue so both
# halves transfer in parallel (SP+ACT HWDGEs) instead of serializing on SP
PAR_INPUT_DMA = False
# raw builder: split the input DMA + delta + first scan step by g-half and
# issue the input DMAs from an early-starting queue. True: g0=ACT g1=SP;
# "act2": both ACT; "gp": both GPSIMD (SWDGE ~1us fixed vs HWDGE ~2.1us)
RAMP_GSPLIT = "act2"
# raw builder: issue the final group's store DMA from the GPSIMD SWDGE.
# Measured WORSE (+3us) on HW — SWDGE issue is slower in practice. Keep off.
FINAL_STORE_GP = False
# raw builder: store the final group per extract-slice. Measured WORSE
# (+2.3us) on HW — every extra store DMA costs ~1us of SP/DGE pipeline.
FINAL_STORE_SPLIT = False
# raw builder: barrier only the engines the program uses (no PE): -0.3us
BARRIER_NO_PE = True
# raw builder: replace the epilogue barrier with a single SP->Pool gate sem.
# Measured WORSE (+0.7us) on HW 2026-08-10 (the drain->sem->Pool hop
# serializes what the butterfly overlaps); re-exec stays safe. Keep off.
EPILOGUE_SLIM = False
# spike store dtype on device: "fp8" (float8_e4m3, 1 byte -- 0.0/1.0 exact,
# host widens via byte != 0) | "bf16" | "f32".
SPIKE_DTYPE = "fp8"

_MODULE_CACHE = {}


def _register_lif_op():
    """Register the fused LIF-step custom DVE op (idempotent)."""
    import concourse.dve_ops as dve_ops
    from concourse.dve_spec import (
        C0, C1, C2, One, Spec, Src0, Src1, _has_src1, lower,
    )
    from concourse.dve_uop import DveOpSpec

    for op in dve_ops.OPS:
        if op.name == _LIF_OP_NAME:
            return op

    def _ref(in0, in1, s0, s1, imm2):
        in0 = np.asarray(in0, np.float32)
        in1 = np.asarray(in1, np.float32)
        keep = (in0 < np.float32(1.0)).astype(np.float32)
        return (
            keep * (in0 * np.float32(imm2))
            + (in1 * np.float32(s0) + np.float32(s1))
        ).astype(np.float32)

    body = (Src0 < One) * (Src0 * C2) + (Src1 * C0 + C1)
    spec = Spec(body=body, reference=_ref)

    row = dve_ops._CUSTOM_DVE_ROW_BASE + len(dve_ops.OPS)
    assert row < 0x20, "no free custom-DVE opcode rows"
    shas = {}
    for ver in ("v3", "v4"):
        uops = lower(spec, ver=ver)
        shas[ver] = DveOpSpec(
            name=_LIF_OP_NAME, opcode=row, uops=uops, rd1_en=_has_src1(spec)
        ).sha(ver)

    op = dve_ops.DveOp(_LIF_OP_NAME, spec, subdim=False, uops_sha=shas)
    dve_ops.OPS.append(op)
    dve_ops._SUB_OPCODE_FOR_NAME[op.name] = row
    dve_ops.CUSTOM_DVE_SPECS[op.name] = spec
    return op


_LIF_SPIKE_OP_NAME = "LIF_STEP_SPIKE_ANT_RT"


def _register_lif_spike_op():
    """Fused final step: spike = (LIF_step(h, d) >= 1), membrane discarded.
    Same inner expression tree as LIF_STEP_ANT_RT (bit-identical h before
    the compare), one extra ALU stage for >= 1."""
    import concourse.dve_ops as dve_ops
    from concourse.dve_spec import (
        C0, C1, C2, One, Spec, Src0, Src1, _has_src1, lower,
    )
    from concourse.dve_uop import DveOpSpec

    for op in dve_ops.OPS:
        if op.name == _LIF_SPIKE_OP_NAME:
            return op

    def _ref(in0, in1, s0, s1, imm2):
        in0 = np.asarray(in0, np.float32)
        in1 = np.asarray(in1, np.float32)
        keep = (in0 < np.float32(1.0)).astype(np.float32)
        h = keep * (in0 * np.float32(imm2)) + (
            in1 * np.float32(s0) + np.float32(s1)
        )
        return (h >= np.float32(1.0)).astype(np.float32)

    body = ((Src0 < One) * (Src0 * C2) + (Src1 * C0 + C1)) >= One
    spec = Spec(body=body, reference=_ref)

    row = dve_ops._CUSTOM_DVE_ROW_BASE + len(dve_ops.OPS)
    assert row < 0x20, "no free custom-DVE opcode rows"
    shas = {}
    for ver in ("v3", "v4"):
        uops = lower(spec, ver=ver)
        shas[ver] = DveOpSpec(
            name=_LIF_SPIKE_OP_NAME, opcode=row, uops=uops,
            rd1_en=_has_src1(spec)
        ).sha(ver)

    op = dve_ops.DveOp(_LIF_SPIKE_OP_NAME, spec, subdim=False, uops_sha=shas)
    dve_ops.OPS.append(op)
    dve_ops._SUB_OPCODE_FOR_NAME[op.name] = row
    dve_ops.CUSTOM_DVE_SPECS[op.name] = spec
    return op


def _spike_groups():
    """Store/extract group sizes along o: SPD-step groups, tapered at the
    end to shrink the kernel tail (small final ACT + DMA)."""
    if GROUPS_OVERRIDE is not None:
        assert sum(GROUPS_OVERRIDE) == _O
        return list(GROUPS_OVERRIDE)
    groups = []
    rem = _O
    while rem > 2 * STEPS_PER_DMA:
        groups.append(STEPS_PER_DMA)
        rem -= STEPS_PER_DMA
    while rem > 0:
        g_sz = rem if rem <= 2 else max(2, rem // 2)
        groups.append(g_sz)
        rem -= g_sz
    return groups


def _build_module(
    spike_engine: str, reps: int = 1, variant: str = "full", imm_coefs=None
):
    """Build the Bass/Tile module (one NeuronCore program, SPMD across 8).

    reps > 1 wraps the whole computation in a device-side loop -- used only
    for differential wall-clock timing (output is rewritten identically).
    variant: "full" | "scanonly" (no spikes/stores) | "nostore" (no DMA out)
             | "noscan" (one memset h + spikes/stores only) -- bench-only.
    """
    import concourse.bacc as bacc
    import concourse.mybir as mybir
    from concourse.tile import TileContext

    lif_op = _register_lif_op()

    nc = bacc.Bacc(
        "TRN2",
        target_bir_lowering=False,
        debug=False,
        enable_asserts=False,
        num_devices=_NC,
    )
    f32 = mybir.dt.float32

    NFREE = _G * _T          # 1024
    groups = _spike_groups()

    x_in = nc.dram_tensor("x_bft", [_BL * _F, _T], f32, kind="ExternalInput").ap()
    a_in = nc.dram_tensor("a_coef", [_P, _O], f32, kind="ExternalInput").ap()
    c_in = nc.dram_tensor("c_coef", [_P, _O], f32, kind="ExternalInput").ap()
    if SPIKE_DTYPE == "fp8":
        sdt = mybir.dt.float8e4
    elif SPIKE_DTYPE == "bf16":
        sdt = mybir.dt.bfloat16
    else:
        sdt = f32
    # blocked layout: [p=(b1,f), o, g, t]; host permutes to [b=2g+b1, o, f, t]
    out = nc.dram_tensor(
        "spikes", [_P, _O, _G, _T], sdt, kind="ExternalOutput"
    ).ap()
    out2 = out.rearrange("p o g t -> p (o g t)")  # [128, O*NFREE], contiguous rows

    with TileContext(nc) as tc:
        with (
            tc.tile_pool(name="const", bufs=1) as cpool,
            tc.tile_pool(name="xd", bufs=1) as xpool,
            tc.tile_pool(name="state", bufs=HPOOL_BUFS) as hpool,
            tc.tile_pool(name="spk", bufs=SPOOL_BUFS) as spool,
        ):

            def body():
                if variant == "empty":
                    z_t = cpool.tile([_P, 1], f32, tag="z")
                    nc.vector.memset(z_t[:], 0.0)
                    return
                a_t = c_t = None
                if imm_coefs is None:
                    a_t = cpool.tile([_P, _O], f32, tag="a")
                    c_t = cpool.tile([_P, _O], f32, tag="c")
                    nc.sync.dma_start(out=a_t[:], in_=a_in[:])
                    nc.sync.dma_start(out=c_t[:], in_=c_in[:])

                x_t = xpool.tile([_P, NFREE], f32, tag="x")
                d_t = xpool.tile([_P, NFREE], f32, tag="d")
                x3 = x_t[:].rearrange("p (g t) -> p g t", g=_G)
                d3 = d_t[:].rearrange("p (g t) -> p g t", g=_G)
                x_src = x_in.rearrange("(g p) t -> p g t", p=_P)
                mset = nc.gpsimd if MEMSET_ENGINE == "gpsimd" else nc.vector
                # delta along t: d[...,0] = 0 ; d[...,1:] = x[...,1:] - x[...,:-1]
                # (input DMA + sub optionally chunked along g for faster rampup)
                if INPUT_SPLIT <= 1:
                    nc.sync.dma_start(out=x3, in_=x_src)
                    mset.memset(d3[:, :, 0:1], 0.0)
                    nc.vector.tensor_sub(
                        out=d3[:, :, 1:_T], in0=x3[:, :, 1:_T],
                        in1=x3[:, :, 0 : _T - 1],
                    )
                else:
                    for gi in range(_G):
                        nc.sync.dma_start(
                            out=x3[:, gi : gi + 1], in_=x_src[:, gi : gi + 1]
                        )
                    mset.memset(d3[:, :, 0:1], 0.0)
                    for gi in range(_G):
                        nc.vector.tensor_sub(
                            out=d3[:, gi : gi + 1, 1:_T],
                            in0=x3[:, gi : gi + 1, 1:_T],
                            in1=x3[:, gi : gi + 1, 0 : _T - 1],
                        )

                sigb = None
                if spike_engine == "act" and variant != "scanonly":
                    sigb = cpool.tile([_P, 1], f32, tag="sigb")
                    mset.memset(sigb[:], -(2.0**100))
                h_zero = None
                if variant == "noscan":
                    # one static h tile reused by every group's extraction
                    h_zero = cpool.tile([_P, STEPS_PER_DMA * NFREE], f32, tag="hz")
                    nc.vector.memset(h_zero[:], 0.0)

                decay = 1.0 - 1.0 / _TAU
                o_base = 0
                h_half = None   # AP of the previous step's h slot
                for g_sz in groups:
                    if variant != "noscan":
                        h_g = hpool.tile([_P, g_sz * NFREE], f32, tag="h")
                        for oi in range(g_sz):
                            o = o_base + oi
                            out_ap = h_g[:, oi * NFREE : (oi + 1) * NFREE]
                            if imm_coefs is not None:
                                s0o, s1o = float(imm_coefs[0][o]), float(imm_coefs[1][o])
                            else:
                                s0o, s1o = a_t[:, o : o + 1], c_t[:, o : o + 1]
                            if o == 0:
                                # v=0: h_0 = d*A_0 + C_0 (2x-mode tensor_scalar,
                                # replaces state memset + first custom op)
                                nc.vector.tensor_scalar(
                                    out_ap,
                                    d_t[:],
                                    s0o,
                                    s1o,
                                    mybir.AluOpType.mult,
                                    mybir.AluOpType.add,
                                )
                            else:
                                nc.vector._custom_dve(
                                    lif_op,
                                    out=out_ap,
                                    in0=h_half,
                                    in1=d_t[:],
                                    s0=s0o,
                                    s1=s1o,
                                    imm2=decay,
                                )
                            h_half = out_ap
                    else:
                        h_g = h_zero
                    if variant == "scanonly":
                        o_base += g_sz
                        continue
                    s_mega = spool.tile([_P, g_sz * NFREE], sdt, tag="s")
                    nsp = min(EXTRACT_SPLIT, g_sz)
                    bounds = [g_sz * i // nsp for i in range(nsp + 1)]
                    for b0, b1 in zip(bounds[:-1], bounds[1:]):
                        s_ap = s_mega[:, b0 * NFREE : b1 * NFREE]
                        h_ap = h_g[:, b0 * NFREE : b1 * NFREE]
                        if spike_engine == "gpsimd":
                            nc.gpsimd.tensor_single_scalar(
                                s_ap, h_ap, 1.0, mybir.AluOpType.is_ge
                            )
                        elif spike_engine == "act":
                            # sigmoid(2^100*(h-1)) saturates to exactly 0/1
                            nc.scalar.activation(
                                s_ap,
                                h_ap,
                                mybir.ActivationFunctionType.Sigmoid,
                                bias=sigb[:],
                                scale=2.0**100,
                            )
                        else:
                            nc.vector.tensor_single_scalar(
                                s_ap, h_ap, 1.0, mybir.AluOpType.is_ge
                            )
                    if variant != "nostore":
                        lo = o_base * NFREE
                        hi = (o_base + g_sz) * NFREE
                        nc.sync.dma_start(out=out2[:, lo:hi], in_=s_mega[:])
                    o_base += g_sz

            if reps == 1:
                body()
            else:
                with tc.For_i(0, reps, 1):
                    body()

    nc.finalize()
    return nc


def _build_module_raw(imm_coefs):
    """No-Tile production builder (act engine, fp8 spikes, imm coefs).

    Identical dataflow to _build_module(variant="full"), but semaphores only
    on true cross-engine edges:
      s_x    input DMA done + delta    -> DVE delta / GP scan start
      s_dve  DVE scan progress         -> ACT extract slices
      s_gp   GP scan progress (planes) -> ACT extract slices
      s_act  extracts done             -> SP store DMA + h-ring reuse
      s_dma[j] store DMA (s-slot j)    -> s-ring slot reuse
    The DVE scan chain itself carries no waits: the engine is in-order and
    each op's first read trails the previous op's first write by a full
    stream (>> SBUF write latency).

    GP_COLS > 0 splits each h plane's free dim: cols [0, D) scan on the
    DVE (custom fused op), cols [D, NFREE) scan on the GPSIMD (Pool)
    engine via 4 plain ops/step. Both write disjoint ranges of the same
    h tile, so ACT extraction and the store DMA see whole planes.
    """
    import contextlib

    import concourse.bacc as bacc
    import concourse.mybir as mybir

    assert imm_coefs is not None
    lif_op = _register_lif_op()
    # only register the fused op when used — keeps the DVE table identical
    # to the hardware-validated build otherwise
    lif_spike_op = _register_lif_spike_op() if RAW_TAIL_FUSED else None

    nc = bacc.Bacc(
        "TRN2",
        target_bir_lowering=False,
        debug=False,
        enable_asserts=False,
        num_devices=_NC,
    )
    f32 = mybir.dt.float32
    sdt = {"fp8": mybir.dt.float8e4, "bf16": mybir.dt.bfloat16,
           "f32": f32}[SPIKE_DTYPE]

    NFREE = _G * _T          # 1024
    GP = GP_COLS
    D = NFREE - GP           # DVE-owned columns per plane
    assert not (RAMP_GSPLIT and GP), "RAMP_GSPLIT writes full-width plane 0"
    groups = _spike_groups()
    if RAMP_GSPLIT:
        # plane 0 is emitted from the prologue without a slice-boundary
        # s_dve inc; the first extract slice must span >= 2 planes
        nsp0 = min(EXTRACT_SPLIT, groups[0])
        assert groups[0] // nsp0 >= 2, "RAMP_GSPLIT needs first slice >= 2"
    ngrp = len(groups)
    NRING = 3                # h / s ring depth
    HMAX = max(groups)

    x_in = nc.dram_tensor("x_bft", [_BL * _F, _T], f32, kind="ExternalInput").ap()
    out = nc.dram_tensor(
        "spikes", [_P, _O, _G, _T], sdt, kind="ExternalOutput"
    ).ap()
    out2 = out.rearrange("p o g t -> p (o g t)")

    s_x = nc.alloc_semaphore("s_x")
    s_x2 = nc.alloc_semaphore("s_x2")
    s_d = nc.alloc_semaphore("s_d") if GP else None
    s_dve = nc.alloc_semaphore("s_dve")
    s_gp = nc.alloc_semaphore("s_gp") if GP else None
    s_act = nc.alloc_semaphore("s_act")
    s_dma = [nc.alloc_semaphore(f"s_dma{j}") for j in range(NRING)]

    with contextlib.ExitStack() as stack:
        xt = stack.enter_context(nc.sbuf_tensor("x", [_P, NFREE], f32))
        dt_ = stack.enter_context(nc.sbuf_tensor("d", [_P, NFREE], f32))
        sgt = stack.enter_context(nc.sbuf_tensor("sigb", [_P, 1], f32))
        hts = [stack.enter_context(
                   nc.sbuf_tensor(f"h{j}", [_P, HMAX * NFREE], f32))
               for j in range(NRING)]
        sts = [stack.enter_context(
                   nc.sbuf_tensor(f"s{j}", [_P, HMAX * NFREE], sdt))
               for j in range(NRING)]
        gp_scr = []
        if GP:
            gp_scr = [stack.enter_context(
                          nc.sbuf_tensor(f"gps{j}", [_P, GP], f32))
                      for j in range(3)]
        x_t, d_t, sigb = xt.ap(), dt_.ap(), sgt.ap()
        h_bufs = [t.ap() for t in hts]
        s_bufs = [t.ap() for t in sts]
        k_t, r_t, e_t = (t.ap() for t in gp_scr) if GP else (None, None, None)

        x3 = x_t.rearrange("p (g t) -> p g t", g=_G)
        d3 = d_t.rearrange("p (g t) -> p g t", g=_G)
        d_dve = d_t[:, 0:D]
        d_gp = d_t[:, D:NFREE] if GP else None

        x_src = x_in.rearrange("(g p) t -> p g t", p=_P)
        TH = _T // 2
        h0_full = None
        if RAMP_GSPLIT:
            # input split by g-half; g0 issues from the ACT HWDGE queue
            # (its sequencer starts ~0.9us before SP's). Delta is per-g
            # along t, so each half's sub + first scan step run as soon as
            # that half lands; the g1 transfer overlaps g0's sub + o=0a.
            eng1 = nc.gpsimd if RAMP_GSPLIT == "gp" else nc.scalar
            eng2 = {"act2": nc.scalar, "gp": nc.gpsimd}.get(
                RAMP_GSPLIT, nc.sync)
            eng1.dma_start(
                out=x3[:, 0:1], in_=x_src[:, 0:1]
            ).then_inc(s_x, 16)
            eng2.dma_start(
                out=x3[:, 1:2], in_=x_src[:, 1:2]
            ).then_inc(s_x2, 16)
            # memsets before the DMA wait: the wait gives their SBUF writes
            # commit slack (same-engine W->R has no interlock)
            nc.vector.memset(d3[:, :, 0:1], 0.0)
            nc.vector.memset(sigb, -(2.0**100))
            s00 = float(imm_coefs[0][0])
            s10 = float(imm_coefs[1][0])
            h_g0 = h_bufs[0]
            nc.vector.wait_ge(s_x, 16)
            nc.vector.tensor_sub(
                out=d3[:, 0:1, 1:_T], in0=x3[:, 0:1, 1:_T],
                in1=x3[:, 0:1, 0 : _T - 1],
            )
            # o=0 on the g0 half (tensor_scalar, 2x_2p) while g1 transfers
            nc.vector.tensor_scalar(
                h_g0[:, 0:_T], d_t[:, 0:_T], s00, s10,
                mybir.AluOpType.mult, mybir.AluOpType.add,
            )
            nc.vector.wait_ge(s_x2, 16)
            nc.vector.tensor_sub(
                out=d3[:, 1:2, 1:_T], in0=x3[:, 1:2, 1:_T],
                in1=x3[:, 1:2, 0 : _T - 1],
            )
            ins = nc.vector.tensor_scalar(
                h_g0[:, _T:NFREE], d_t[:, _T:NFREE], s00, s10,
                mybir.AluOpType.mult, mybir.AluOpType.add,
            )
            h0_full = h_g0[:, 0:NFREE]
        else:
            # input DMA split along t; with PAR_INPUT_DMA the second half
            # issues from the ACT HWDGE so both halves transfer concurrently
            nc.sync.dma_start(
                out=x3[:, :, 0:TH], in_=x_src[:, :, 0:TH]
            ).then_inc(s_x, 16)
            eng2 = nc.scalar if PAR_INPUT_DMA else nc.sync
            eng2.dma_start(
                out=x3[:, :, TH:_T], in_=x_src[:, :, TH:_T]
            ).then_inc(s_x2, 16)

            # DVE prologue. Both memsets go FIRST, before the input-DMA
            # wait: the engine has no same-engine write->read interlock, so
            # a short producer's SBUF write (~58-116 cycles to commit) must
            # be given slack before its consumer's first read. The multi-us
            # DMA wait provides it. (The LIF chain needs no such care: op
            # k+1's read of element i trails op k's write by a full stream.)
            nc.vector.memset(d3[:, :, 0:1], 0.0)
            nc.vector.memset(sigb, -(2.0**100))
            nc.vector.wait_ge(s_x, 16)
            nc.vector.tensor_sub(
                out=d3[:, :, 1:TH], in0=x3[:, :, 1:TH],
                in1=x3[:, :, 0 : TH - 1],
            )
            nc.vector.wait_ge(s_x2, 16)
            # second half includes the boundary column t=TH (reads x[TH-1]
            # from the first half's region — cross-DMA, both sem-gated)
            ins = nc.vector.tensor_sub(
                out=d3[:, :, TH:_T], in0=x3[:, :, TH:_T],
                in1=x3[:, :, TH - 1 : _T - 1],
            )
        if GP:
            # d fully materialized -> GP scan may start (dedicated sem: the
            # input-DMA incs on s_x would satisfy a combined threshold early)
            ins.then_inc(s_d, 1)
            nc.gpsimd.wait_ge(s_d, 1)

        # static schedule bookkeeping
        decay = 1.0 - 1.0 / _TAU
        dve_incs = 0          # s_dve increments issued so far
        act_incs = 0          # s_act increments issued so far
        dma_cnt = [0] * NRING  # store DMAs issued per s-ring slot
        # per-group precomputed: extract slice bounds
        o_base = 0
        h_half = None         # DVE carry AP (cols [0:D) of prev plane)
        gp_half = None        # GP carry AP (cols [D:NFREE) of prev plane)
        acts_after_group = []
        alu = mybir.AluOpType
        for gi, g_sz in enumerate(groups):
            h_g = h_bufs[gi % NRING]
            s_g = s_bufs[gi % NRING]
            nsp = min(EXTRACT_SPLIT, g_sz)
            bounds = [g_sz * i // nsp for i in range(nsp + 1)]
            if gi >= NRING:
                # h-ring reuse: ACT must have finished reading group gi-NRING
                nc.vector.wait_ge(s_act, acts_after_group[gi - NRING])
                if GP:
                    nc.gpsimd.wait_ge(s_act, acts_after_group[gi - NRING])
            for oi in range(g_sz):
                o = o_base + oi
                s0o = float(imm_coefs[0][o])
                s1o = float(imm_coefs[1][o])
                out_ap = h_g[:, oi * NFREE : oi * NFREE + D]
                if o == 0 and RAMP_GSPLIT:
                    # plane 0 was produced by the prologue's two g-half
                    # tensor_scalar ops (into this same ring slot)
                    h_half = h0_full
                    continue
                if o == 0:
                    ins = nc.vector.tensor_scalar(
                        out_ap, d_dve, s0o, s1o, alu.mult, alu.add,
                    )
                else:
                    ins = nc.vector._custom_dve(
                        lif_op, out=out_ap, in0=h_half, in1=d_dve,
                        s0=s0o, s1=s1o, imm2=decay,
                    )
                h_half = out_ap
                if oi + 1 in bounds:
                    ins.then_inc(s_dve, 1)
                    dve_incs += 1
                if GP:
                    gp_out = h_g[:, oi * NFREE + D : (oi + 1) * NFREE]
                    if o == 0:
                        gins = nc.gpsimd.tensor_scalar(
                            gp_out, d_gp, s0o, s1o, alu.mult, alu.add,
                        )
                    else:
                        # k = 0.5*[h<1]; r = k*h; e = d*A+C; h' = r + e
                        nc.gpsimd.tensor_scalar(
                            k_t, gp_half, 1.0, decay, alu.is_lt, alu.mult,
                        )
                        nc.gpsimd.tensor_tensor(
                            out=r_t, in0=k_t, in1=gp_half, op=alu.mult,
                        )
                        nc.gpsimd.tensor_scalar(
                            e_t, d_gp, s0o, s1o, alu.mult, alu.add,
                        )
                        gins = nc.gpsimd.tensor_tensor(
                            out=gp_out, in0=r_t, in1=e_t, op=alu.add,
                        )
                    gp_half = gp_out
                    gins.then_inc(s_gp, 1)  # planes completed (global o+1)
            # ACT extraction for this group, slice by slice
            for si, (b0, b1) in enumerate(zip(bounds[:-1], bounds[1:])):
                on_dve = (
                    RAW_TAIL_DVE and gi == ngrp - 1 and si == nsp - 1
                )
                if on_dve:
                    # final slice: extract on DVE (and GP for its columns)
                    # right after the last LIF step (program order, no sem
                    # hop; fp8 out). WAR guard: this slot's previous store
                    # DMA must be done (satisfied long before; free).
                    nc.vector.wait_ge(
                        s_dma[gi % NRING], 16 * dma_cnt[gi % NRING]
                    )
                    n_pl = b1 - b0
                    if GP:
                        sl_s = s_g[:, b0 * NFREE : b1 * NFREE].rearrange(
                            "p (n f) -> p n f", n=n_pl)
                        sl_h = h_g[:, b0 * NFREE : b1 * NFREE].rearrange(
                            "p (n f) -> p n f", n=n_pl)
                        nc.vector.tensor_single_scalar(
                            sl_s[:, :, 0:D], sl_h[:, :, 0:D],
                            1.0, alu.is_ge,
                        ).then_inc(s_act, 1)
                        act_incs += 1
                        nc.gpsimd.wait_ge(
                            s_dma[gi % NRING], 16 * dma_cnt[gi % NRING]
                        )
                        nc.gpsimd.tensor_single_scalar(
                            sl_s[:, :, D:NFREE], sl_h[:, :, D:NFREE],
                            1.0, alu.is_ge,
                        ).then_inc(s_act, 1)
                        act_incs += 1
                    else:
                        nc.vector.tensor_single_scalar(
                            s_g[:, b0 * NFREE : b1 * NFREE],
                            h_g[:, b0 * NFREE : b1 * NFREE],
                            1.0, alu.is_ge,
                        ).then_inc(s_act, 1)
                        act_incs += 1
                    continue
                # progress target: all slices up to b1 of this group done
                done_slices = dve_incs - (nsp - 1 - si)
                nc.scalar.wait_ge(s_dve, done_slices)
                if GP:
                    nc.scalar.wait_ge(s_gp, o_base + b1)
                if si == 0 and gi >= NRING:
                    # s-ring reuse: slot's previous store DMA must be done
                    nc.scalar.wait_ge(
                        s_dma[gi % NRING], 16 * dma_cnt[gi % NRING]
                    )
                nc.scalar.activation(
                    s_g[:, b0 * NFREE : b1 * NFREE],
                    h_g[:, b0 * NFREE : b1 * NFREE],
                    mybir.ActivationFunctionType.Sigmoid,
                    bias=sigb,
                    scale=2.0**100,
                ).then_inc(s_act, 1)
                act_incs += 1
            acts_after_group.append(act_incs)
            # store DMA for the whole group (final group optionally via the
            # GPSIMD SWDGE — lower issue latency, shorter tail)
            st_eng = (nc.gpsimd if (FINAL_STORE_GP and gi == ngrp - 1)
                      else nc.sync)
            if FINAL_STORE_SPLIT and gi == ngrp - 1 and nsp > 1:
                # per-slice stores: earlier slices' transfers overlap the
                # final slice's extraction; the last transfer shrinks
                for si, (b0, b1) in enumerate(zip(bounds[:-1], bounds[1:])):
                    st_eng.wait_ge(
                        s_act, act_incs - (nsp - 1 - si) - (1 if GP else 0))
                    st_eng.dma_start(
                        out=out2[:, (o_base + b0) * NFREE
                                 : (o_base + b1) * NFREE],
                        in_=s_g[:, b0 * NFREE : b1 * NFREE],
                    ).then_inc(s_dma[gi % NRING], 16)
                    dma_cnt[gi % NRING] += 1
                o_base += g_sz
                continue
            st_eng.wait_ge(s_act, act_incs)
            lo = o_base * NFREE
            hi = (o_base + g_sz) * NFREE
            st_eng.dma_start(
                out=out2[:, lo:hi], in_=s_g[:, : g_sz * NFREE]
            ).then_inc(s_dma[gi % NRING], 16)
            dma_cnt[gi % NRING] += 1
            o_base += g_sz

        # end of program: every store DMA landed, then reset semaphores so a
        # re-execution of the same NEFF starts from zero. One barrier before
        # the clear (every engine is provably past its final sem wait — a
        # clear racing a polling waiter would deadlock); no second barrier:
        # re-execution is gated on full queue drain, which includes the
        # gpsimd clear itself.
        if RAW_EPILOGUE and EPILOGUE_SLIM:
            # SP's final s_dma waits prove every engine's program (and thus
            # every sem wait) has completed: DVE/ACT end strictly before the
            # final store's gating s_act count is reached. One SP->Pool gate
            # sem then orders the clears; no engine can still be polling.
            s_gate = nc.alloc_semaphore("s_gate")
            for j in range(NRING):
                nc.sync.wait_ge(s_dma[j], 16 * dma_cnt[j])
            nc.sync.drain().then_inc(s_gate, 1)
            nc.gpsimd.wait_ge(s_gate, 1)
            sems = [s_x, s_x2, s_dve, s_act, *s_dma, s_gate]
            if s_gp is not None:
                sems.extend([s_gp, s_d])
            nc.clear_and_free_semaphores(sems)
        else:
            for j in range(NRING):
                nc.sync.wait_ge(s_dma[j], 16 * dma_cnt[j])
            nc.sync.drain()
            if RAW_EPILOGUE:
                if BARRIER_NO_PE:
                    import concourse.mybir as _mb
                    nc.multi_engine_barrier(
                        [_mb.EngineType.SP, _mb.EngineType.DVE,
                         _mb.EngineType.Activation, _mb.EngineType.Pool])
                else:
                    nc.all_engine_barrier()
                sems = [s_x, s_x2, s_dve, s_act, *s_dma]
                if s_gp is not None:
                    sems.extend([s_gp, s_d])
                nc.clear_and_free_semaphores(sems)

    nc.finalize()
    return nc


def _get_module(spike_engine: str, imm_coefs=None):
    raw = RAW and spike_engine == "act" and imm_coefs is not None
    if imm_coefs is not None:
        key = (spike_engine, SPIKE_DTYPE, raw, GP_COLS, PAR_INPUT_DMA,
               str(RAMP_GSPLIT), FINAL_STORE_GP, FINAL_STORE_SPLIT,
               BARRIER_NO_PE, EPILOGUE_SLIM,
               imm_coefs[0].tobytes(), imm_coefs[1].tobytes())
    else:
        key = (spike_engine, SPIKE_DTYPE, raw)
    if key not in _MODULE_CACHE:
        if raw:
            _MODULE_CACHE[key] = _build_module_raw(imm_coefs)
        else:
            _MODULE_CACHE[key] = _build_module(spike_engine, imm_coefs=imm_coefs)
    return _MODULE_CACHE[key]


def _prepare_inputs(inputs, enc_w, enc_b, bn_w, bn_b, bn_mean, bn_var):
    """Host-side marshalling: scalar folding + per-core shard/transpose."""
    x = np.ascontiguousarray(np.asarray(inputs, np.float32))
    w = np.asarray(enc_w, np.float32).reshape(_O)
    b = np.asarray(enc_b, np.float32).reshape(_O)
    bw = np.float64(np.asarray(bn_w).reshape(())[()])
    bb = np.float64(np.asarray(bn_b).reshape(())[()])
    bm = np.float64(np.asarray(bn_mean).reshape(())[()])
    bv = np.float64(np.asarray(bn_var).reshape(())[()])

    inv = bw / np.sqrt(bv + _EPS)
    beta = bb - bm * inv
    A = (inv * w.astype(np.float64) / _TAU).astype(np.float32)
    C = (((beta * w.astype(np.float64)) + b.astype(np.float64)) / _TAU).astype(
        np.float32
    )
    a_b = np.ascontiguousarray(np.broadcast_to(A, (_P, _O)))
    c_b = np.ascontiguousarray(np.broadcast_to(C, (_P, _O)))

    in_maps = []
    for core in range(_NC):
        xc = x[core * _BL : (core + 1) * _BL]          # [4, T, F]
        xt = np.ascontiguousarray(xc.transpose(0, 2, 1)).reshape(_BL * _F, _T)
        in_maps.append({"x_bft": xt, "a_coef": a_b, "c_coef": c_b})
    return in_maps


def _to_f32_spikes(v: np.ndarray) -> np.ndarray:
    """Device spike array -> f32 0.0/1.0 (exact: spike encodings are
    0x00 vs nonzero in every supported dtype; sigmoid emits +0.0 only)."""
    v = np.asarray(v)
    if v.dtype == np.float32:
        return v
    if v.itemsize == 1:
        return (v.view(np.uint8) != 0).astype(np.float32)
    if v.itemsize == 2:
        return (v.view(np.uint16) != 0).astype(np.float32)
    raise ValueError(f"unexpected spike dtype {v.dtype}")


def _unpack_core(spk_blocked: np.ndarray) -> np.ndarray:
    """[p=(b1,f), o, g, t] -> [b=2g+b1, o, f, t] (widened to f32)."""
    v = _to_f32_spikes(spk_blocked)
    v = v.reshape(2, _F, _O, _G, _T)                     # [b1, f, o, g, t]
    v = v.transpose(3, 0, 2, 1, 4)                       # [g, b1, o, f, t]
    return np.ascontiguousarray(v.reshape(_BL, _O, _F, _T))


def _run(in_maps, spike_engine=None, **spmd_kwargs):
    from concourse.bass_utils import run_bass_kernel_spmd

    eng = spike_engine or SPIKE_ENGINE
    imm_coefs = None
    if USE_IMM:
        imm_coefs = (in_maps[0]["a_coef"][0], in_maps[0]["c_coef"][0])
    nc = _get_module(eng, imm_coefs)
    return run_bass_kernel_spmd(nc, in_maps, core_ids=list(range(_NC)), **spmd_kwargs)


def kernel(inputs, enc_w, enc_b, bn_w, bn_b, bn_mean, bn_var):
    in_maps = _prepare_inputs(inputs, enc_w, enc_b, bn_w, bn_b, bn_mean, bn_var)
    res = _run(in_maps)
    out = np.concatenate([_unpack_core(r["spikes"]) for r in res.results], axis=0)
    return np.ascontiguousarray(out.astype(np.float32, copy=False))

